# revision 41
# baseline (speedup 1.0000x reference)
"""ANI angular symmetry function on 8 TRN2 NeuronCores (Bass/Tile).

out[t, r*4+s] = exp(-ETA*(m-shift_r)^2) * 2*((1+cos(theta-sigma_s))/2)**ZETA
                * fcut(d0)*fcut(d1),  m=(d0+d1)/2, theta=acos(0.95*cos_angle)

Data-parallel over triples T: each core gets T/8 rows (padded).

v3 design (log-space factorization + bf16 2x outer + software pipelining):
  dot   = sum(v0*v1)                               DVE mult + reduce
  t     = 0.95*dot/sqrt((d0*d1)^2-(0.95*dot)^2)    Square/Ln/Exp
  phi   = atan(t);  cos(theta-sigma_s) = sin(phi+sigma_s)
  radial split around center c: exp(-ETA(m-s_r)^2) = E0 * exp(a_r*msum+b_r)
    E0 = exp(-ETA*m'^2) folded (log-space) into the angular factor:
    P4_s = exp(ZETA*ln(0.5c_s+0.5) + lnfcq - ETA*m'^2)        [bf16]
    R2_r = exp(a_r*msum + b_r) written as DUPLICATED PAIRS     [bf16]
  outer: out[f,r,s] = R2[f,r]*P4[f,s]; all-bf16 step-1 innermost APs
    (pairs) so TENSOR_TENSOR runs in 2x_1p mode; 2 TTs per chunk.
Emission is software-pipelined: p1(i+1) is emitted BEFORE outer(i) so the
in-order DVE queue overlaps the next super's dot-products with the current
super's ACT-heavy radial/LHP block, and the outer product lands right when
the P4 exps complete. Only two ACT table sets (natural_log_exp +
trig_and_small); set-bound ACT ops chained in program order.
"""
import math
import os
import numpy as np

ETA = 12.5
ZETA = 14.1
CUTOFF = 3.5
NCORES = 8
P = 128
CENTER = 2.1

NO_CHAIN = os.environ.get("ANI_NO_CHAIN", "0") == "1"
GPS_ARG = os.environ.get("ANI_GPS_ARG", "0") == "1"  # arg-STT on gpsimd
SQDVE = os.environ.get("ANI_SQDVE", "0") == "1"   # u^2/dot^2 on DVE
W13DVE = os.environ.get("ANI_W13DVE", "1") == "1"  # odd-section sums on DVE

_cache = {}
last_results = None  # BassKernelResults from the most recent run (for test.py)


def _geometry(T):
    per = (T + NCORES - 1) // NCORES
    # f_tot multiple of 120: supers with F_S % 40 == 0 (outer chunks of
    # F_S/10 and v-chunks of F_S/8 both integral)
    f_tot = ((per + P * 120 - 1) // (P * 120)) * 120
    return per, f_tot


def _split_supers(f_tot):
    # 4 supers, all % 40 == 0; first and last smallest (shorter DMA ramp
    # and pipeline tail)
    base = f_tot // 4 // 40 * 40
    rest = f_tot - 4 * base
    sizes = [base] * 4
    i = 1
    while rest > 0:
        sizes[i] += 40
        rest -= 40
        i = i + 1 if i < 2 else 1
    return sizes


def _build(shifts, sections, f_tot):
    import concourse.bass as bass
    import concourse.bacc as bacc
    import concourse.tile as tile
    from concourse import mybir
    from concourse.tile_rust import add_dep_helper

    AF = mybir.ActivationFunctionType
    ALU = mybir.AluOpType
    F32 = mybir.dt.float32
    BF16 = mybir.dt.bfloat16

    sup_sizes = _split_supers(f_tot)
    NSUP = len(sup_sizes)
    assert sum(sup_sizes) == f_tot and all(x % 40 == 0 for x in sup_sizes)
    npad = P * f_tot

    SQS = math.sqrt(ETA) / 2.0
    MQ_BIAS = -math.sqrt(ETA) * CENTER
    sig = [float(x) for x in sections]
    # radial: exp(-ETA(m-s_r)^2) = exp(-ETA m'^2) * exp(a_r*msum + b_r)
    # with m = msum/2, m' = m - CENTER, delta_r = s_r - CENTER;
    # ln(0.5) folds the overall normalization into b_r.
    deltas = [float(s) - CENTER for s in shifts]
    a_r = [ETA * d for d in deltas]
    b_r = [-2.0 * ETA * d * CENTER - ETA * d * d + math.log(0.5)
           for d in deltas]

    # Force ACT table-set selection to the two sets that cover all our
    # functions; the rust pass greedily picks the first set containing a
    # func (ln->natural_log, exp->exp_and_others) which doubles the loads.
    import concourse.bacc as _bacc_mod
    from concourse.hw_specs import get_activation_tables as _real_tabs
    _KEEP = {"natural_log_exp_and_others", "trig_and_small"}

    def _filtered_tabs(arch):
        real = _real_tabs(arch)
        if not _KEEP.issubset(real.keys()):
            return real  # unexpected act_info layout: don't filter
        return {name: (fns if name in _KEEP else set())
                for name, fns in real.items()}

    _bacc_mod.get_activation_tables = _filtered_tabs

    nc = bacc.Bacc("TRN2", target_bir_lowering=False, debug=False,
                   num_devices=NCORES)

    d_dram = nc.dram_tensor("d", [2, npad], F32, kind="ExternalInput")
    v_dram = nc.dram_tensor("v", [2, npad, 3], F32, kind="ExternalInput")
    o_dram = nc.dram_tensor("out", [npad, 32], BF16, kind="ExternalOutput")

    d_v = [d_dram[i].rearrange("(p f) -> p f", p=P) for i in range(2)]
    v_v = [v_dram[i].rearrange("(p f) c -> p (f c)", p=P) for i in range(2)]
    o_v = o_dram.rearrange("(p f) k -> p (f k)", p=P)

    # chain all table-set-bound ACT ops in program order
    prev_act = [None]
    SET_BOUND = {AF.Ln, AF.Exp, AF.Sin, AF.Arctan}

    def act(out, in_, func, **kw):
        ins = nc.scalar.activation(out, in_, func, **kw)
        if func in SET_BOUND and not NO_CHAIN:
            if prev_act[0] is not None:
                add_dep_helper(ins.ins, prev_act[0].ins, sync=False,
                               reason="act-set-order")
            prev_act[0] = ins
        return ins

    with tile.TileContext(nc) as tc:
        import contextlib
        ctx = contextlib.ExitStack()
        with ctx:
            consts = ctx.enter_context(tc.tile_pool(name="consts", bufs=1))
            pers2 = ctx.enter_context(tc.tile_pool(name="pers2", bufs=2))
            po1 = ctx.enter_context(tc.tile_pool(name="po1", bufs=1))
            rp2 = ctx.enter_context(tc.tile_pool(name="rp2", bufs=2))
            pv = ctx.enter_context(tc.tile_pool(name="pv", bufs=2))
            op = ctx.enter_context(tc.tile_pool(name="op", bufs=3))

            cvals = [math.log(0.95), 0.5, math.pi / 2, MQ_BIAS, 0.0,
                     sig[0], math.pi / 2 - sig[0]]
            cvals += b_r
            cb = consts.tile([P, len(cvals)], F32, name="cb")
            for i, v in enumerate(cvals):
                nc.vector.memset(cb[:, i:i + 1], v)
            (B_LN095, B_HALF, B_PI2, B_MQ, B_ZERO) = (
                cb[:, i:i + 1] for i in range(5))
            B_SIGA = cb[:, 5:6]   # sigma_0
            B_SIGB = cb[:, 6:7]   # pi/2 - sigma_0 (scale=-1 identity)
            B_RAD = [cb[:, 7 + i:8 + i] for i in range(8)]

            def emit_p1(sc, j_sup, F_S, tiles):
                """dot, t = cot(theta), msum for super sc."""
                F1 = F_S // 2
                d_sb = pers2.tile([P, 2 * F_S], F32, name="d_sb", tag="d_sb")
                nc.sync.dma_start(d_sb[:, :F_S], d_v[0][:, j_sup:j_sup + F_S])
                nc.sync.dma_start(d_sb[:, F_S:], d_v[1][:, j_sup:j_sup + F_S])
                t_sb = pers2.tile([P, F_S], F32, name="t_sb", tag="t_sb")
                msum = pers2.tile([P, F_S], F32, name="msum", tag="msum")
                dot = po1.tile([P, F_S], F32, name="dot", tag="dot")
                q2 = po1.tile([P, F_S], F32, name="q2", tag="q2")
                for i1 in range(2):
                    a, b = i1 * F1, (i1 + 1) * F1
                    v0t = pv.tile([P, 3 * F1], F32, name="v0t", tag="v0t")
                    v1t = pv.tile([P, 3 * F1], F32, name="v1t", tag="v1t")
                    nc.sync.dma_start(
                        v0t[:], v_v[0][:, 3 * (j_sup + a):3 * (j_sup + b)])
                    nc.sync.dma_start(
                        v1t[:], v_v[1][:, 3 * (j_sup + a):3 * (j_sup + b)])
                    nc.vector.tensor_tensor(v0t[:], v0t[:], v1t[:], ALU.mult)
                    nc.vector.tensor_reduce(
                        dot[:, a:b], v0t[:].rearrange("p (f c) -> p f c", c=3),
                        axis=mybir.AxisListType.X, op=ALU.add)
                d0s, d1s = d_sb[:, :F_S], d_sb[:, F_S:]
                # first super: half-grained so ACT engages before the full
                # v-DMA lands (shorter pipeline ramp)
                nh = 2 if sc == 0 else 1
                FN = F_S // nh
                for h in range(nh):
                    sl = slice(h * FN, (h + 1) * FN)
                    nc.vector.tensor_tensor(t_sb[:, sl], d0s[:, sl],
                                            d1s[:, sl], ALU.mult)  # u
                    nc.vector.tensor_tensor(msum[:, sl], d0s[:, sl],
                                            d1s[:, sl], ALU.add)
                    if SQDVE:
                        nc.vector.tensor_tensor(t_sb[:, sl], t_sb[:, sl],
                                                t_sb[:, sl], ALU.mult)
                        nc.vector.scalar_tensor_tensor(
                            q2[:, sl], dot[:, sl], 0.9025, dot[:, sl],
                            ALU.mult, ALU.mult)
                    else:
                        nc.scalar.activation(t_sb[:, sl], t_sb[:, sl],
                                             AF.Square, bias=B_ZERO)
                        nc.scalar.activation(q2[:, sl], dot[:, sl],
                                             AF.Square, bias=B_ZERO,
                                             scale=0.95)
                    # S2 = u^2 - (0.95 dot)^2  (in t_sb)
                    nc.vector.tensor_tensor(t_sb[:, sl], t_sb[:, sl],
                                            q2[:, sl], ALU.subtract)
                    act(t_sb[:, sl], t_sb[:, sl], AF.Ln, bias=B_ZERO)
                    act(t_sb[:, sl], t_sb[:, sl], AF.Exp, bias=B_LN095,
                        scale=-0.5)
                    # t = cot(theta) = 0.95*dot/sqrt(S2)
                    nc.vector.tensor_tensor(t_sb[:, sl], dot[:, sl],
                                            t_sb[:, sl], ALU.mult)
                tiles[sc] = (d_sb, t_sb, msum)

            def emit_mid_head(sc, F_S, tiles, mids):
                """trig ladder + fcq chain (independent of the deferred
                outer product, so it precedes it in every queue)."""
                d_sb, t_sb, msum = tiles[sc]
                sins = po1.tile([P, F_S, 4], F32, name="sins", tag="sins")
                P4 = rp2.tile([P, F_S, 4], BF16, name="P4", tag="P4")
                R2 = rp2.tile([P, F_S, 16], BF16, name="R2", tag="R2")
                mids[sc] = (P4, R2)
                # ---- trig set ----
                # fcut cosines first (only needs d_sb) so the fcq ops can
                # start while the Arctan/Sin ladder runs; fcq -> d_sb[:, :F]
                act(d_sb[:], d_sb[:], AF.Sin, bias=B_PI2,
                    scale=-math.pi / CUTOFF)
                act(t_sb[:], t_sb[:], AF.Arctan, bias=B_ZERO)
                # A = sin(phi+sig0) -> slot 0, B = cos(phi+sig0) -> slot 2;
                # slots 1/3 via angle addition:
                #   c1 = (A+B)*sqrt(2)/2, c3 = (B-A)*sqrt(2)/2
                # (the sqrt(2)/2 factor folds into the odd-slot Ln scale)
                act(sins[:, :, 0], t_sb[:], AF.Sin, bias=B_SIGA, scale=1.0)
                act(sins[:, :, 2], t_sb[:], AF.Sin, bias=B_SIGB, scale=-1.0)
                # mq = ETA*m'^2 -> t_sb (square is in every table set);
                # emitted after the Sins (last readers of t_sb).
                nc.scalar.activation(t_sb[:], msum[:], AF.Square, bias=B_MQ,
                                     scale=SQS)
                tiles[sc] = (d_sb, t_sb, msum, sins)

            def emit_mid_head_dve(sc, F_S, tiles):
                """fcq chain + odd-section sums (emitted between outer
                chunks of the previous super so the in-order DVE queue
                matches ready-time order)."""
                d_sb, t_sb, msum, sins = tiles[sc]
                # fcq = (1+c0)*(1+c1)  [= 4*fcut0*fcut1] -> d_sb[:, :F]
                fcq = d_sb[:, :F_S]
                nc.vector.tensor_scalar_add(d_sb[:, F_S:], d_sb[:, F_S:], 1.0)
                nc.vector.scalar_tensor_tensor(
                    fcq, fcq, 1.0, d_sb[:, F_S:], ALU.add, ALU.mult)
                w_eng = nc.gpsimd if not W13DVE else nc.vector
                w_eng.tensor_tensor(sins[:, :, 1], sins[:, :, 0],
                                    sins[:, :, 2], ALU.add)
                w_eng.tensor_tensor(sins[:, :, 3], sins[:, :, 2],
                                    sins[:, :, 0], ALU.subtract)

            def emit_mid_tail(sc, F_S, tiles, mids, split_rad=False):
                """ln/exp block: lnfcq, LHP lns, radials, args, P4 exps."""
                d_sb, t_sb, msum, sins = tiles[sc]
                P4, R2 = mids[sc]
                fcq = d_sb[:, :F_S]
                # Order: lnfcq, all Lns, then radials (no DVE deps) so ACT
                # stays busy while DVE finishes the outer and computes args.
                act(fcq, fcq, AF.Ln, bias=B_ZERO)
                FHW = F_S // 2
                SC_ODD = 0.5 * math.sqrt(0.5)
                sfl = sins[:].rearrange("p f s -> p (f s)")
                pfl = P4[:].rearrange("p f s -> p (f s)")
                for hh in range(2):
                    ha = hh * FHW
                    for par, sc_ in ((0, 0.5), (1, SC_ODD)):
                        lap = bass.AP(tensor=sfl.tensor,
                                      offset=sfl.offset + 4 * ha + par,
                                      ap=[sfl.ap[0], [4, FHW], [2, 2]])
                        act(lap, lap, AF.Ln, bias=B_HALF, scale=sc_)
                # G = ln(fcq) - ETA*m'^2
                nc.vector.scalar_tensor_tensor(
                    fcq, t_sb[:], -1.0, fcq, ALU.mult, ALU.add)

                def emit_rad(ha, n):
                    # radial pairs: R2[f, 2r:2r+2] = exp(a_r*msum + b_r)
                    rfl = R2[:].rearrange("p f k -> p (f k)")
                    for r in range(8):
                        dst = bass.AP(tensor=rfl.tensor,
                                      offset=rfl.offset + 16 * ha + 2 * r,
                                      ap=[rfl.ap[0], [16, n], [1, 2]])
                        src = bass.AP(tensor=msum.tensor,
                                      offset=msum[:].offset + ha,
                                      ap=[msum[:].ap[0], [1, n], [0, 2]])
                        act(dst, src, AF.Exp, bias=B_RAD[r], scale=a_r[r])

                def emit_arg(hh):
                    ha = hh * FHW
                    sq3 = sins[:, ha:ha + FHW, :]
                    gb = d_sb[:, ha:ha + 1]
                    g_ap = bass.AP(tensor=gb.tensor, offset=gb.offset,
                                   ap=[gb.ap[0], [1, FHW], [0, 4]])
                    arg_eng = nc.gpsimd if GPS_ARG else nc.vector
                    arg_eng.scalar_tensor_tensor(
                        sq3, sq3, ZETA, g_ap, ALU.mult, ALU.add)

                def emit_exp(hh, n):
                    ha = hh * n
                    act(pfl[:, 4 * ha:4 * (ha + n)],
                        sfl[:, 4 * ha:4 * (ha + n)], AF.Exp, bias=B_ZERO)

                emit_arg(0)
                emit_arg(1)
                if split_rad:
                    # last super: half-grained radials + quarter-grained P4
                    # exps so the outer product (and its DMA) starts
                    # mid-block, shrinking the pipeline tail
                    FQ = F_S // 4
                    emit_rad(0, FHW)
                    emit_exp(0, FQ)
                    emit_exp(1, FQ)
                    emit_rad(FHW, FHW)
                    emit_exp(2, FQ)
                    emit_exp(3, FQ)
                else:
                    emit_rad(0, F_S)
                    emit_exp(0, FHW)
                    emit_exp(1, FHW)

            def emit_outer(sc, j_sup, F_S, P4, R2, chunks=None, fine=False):
                """all-bf16 2x outer product + DMA out."""
                NCH = 10 if fine else 5
                F3 = F_S // NCH
                rfl = R2[:].rearrange("p f k -> p (f k)")
                pfl = P4[:].rearrange("p f s -> p (f s)")
                for i3 in (range(NCH) if chunks is None else chunks):
                    a = i3 * F3
                    ot = op.tile([P, F3 * 32], BF16, name="ot", tag="ot")
                    ob = ot[:]
                    for sp in range(2):
                        o_ap = bass.AP(tensor=ob.tensor,
                                       offset=ob.offset + 2 * sp,
                                       ap=[ob.ap[0], [32, F3], [4, 8],
                                           [1, 2]])
                        p_ap = bass.AP(tensor=pfl.tensor,
                                       offset=pfl.offset + 4 * a + 2 * sp,
                                       ap=[pfl.ap[0], [4, F3], [0, 8],
                                           [1, 2]])
                        r_ap = bass.AP(tensor=rfl.tensor,
                                       offset=rfl.offset + 16 * a,
                                       ap=[rfl.ap[0], [16, F3], [2, 8],
                                           [1, 2]])
                        nc.vector.tensor_tensor(o_ap, r_ap, p_ap, ALU.mult)
                    g0 = 32 * (j_sup + a)
                    # issue output stores via idle GPSIMD (SWDGE) so they
                    # don't serialize behind input-load issue on the sync
                    # sequencer
                    odma = (nc.sync if os.environ.get("ANI_ODMA") == "s"
                            else nc.gpsimd)
                    odma.dma_start(o_v[:, g0:g0 + 32 * F3], ot[:])

            tiles = {}
            mids = {}
            j_sup = 0
            offs = []
            for sc, fs in enumerate(sup_sizes):
                offs.append(j_sup)
                emit_p1(sc, j_sup, fs, tiles)
                emit_mid_head(sc, fs, tiles, mids)
                if sc > 0:
                    P4p, R2p = mids[sc - 1]
                    emit_outer(sc - 1, offs[sc - 1], sup_sizes[sc - 1],
                               P4p, R2p, chunks=range(3))
                emit_mid_head_dve(sc, fs, tiles)
                if sc > 0:
                    emit_outer(sc - 1, offs[sc - 1], sup_sizes[sc - 1],
                               P4p, R2p, chunks=range(3, 5))
                emit_mid_tail(sc, fs, tiles, mids,
                              split_rad=(sc == NSUP - 1))
                j_sup += fs
            last = NSUP - 1
            emit_outer(last, offs[last], sup_sizes[last], *mids[last],
                       fine=True)

    nc.compile()
    return nc


def kernel(tri_distances, tri_vectors, shifts, sections):
    from concourse.bass_utils import run_bass_kernel_spmd

    T = tri_distances.shape[1]
    per, f_tot = _geometry(T)
    npad = P * f_tot

    key = (T, NO_CHAIN, GPS_ARG, SQDVE, W13DVE, os.environ.get('ANI_ODMA'))
    if key not in _cache:
        _cache[key] = _build(np.asarray(shifts, np.float64),
                             np.asarray(sections, np.float64), f_tot)
    nc = _cache[key]

    d_full = np.ascontiguousarray(np.asarray(tri_distances, np.float32))
    v_full = np.ascontiguousarray(np.asarray(tri_vectors, np.float32))

    in_maps = []
    for i in range(NCORES):
        lo = i * per
        hi = min(lo + per, T)
        n = hi - lo
        dpad = np.empty((2, npad), np.float32)
        dpad[:, :n] = d_full[:, lo:hi]
        dpad[:, n:] = 1.0
        vpad = np.empty((2, npad, 3), np.float32)
        vpad[:, :n] = v_full[:, lo:hi]
        vpad[:, n:, 0] = 1.0
        vpad[:, n:, 1:] = 0.0
        in_maps.append({"d": dpad, "v": vpad})

    trace = os.environ.get("ANI_TRACE", "0") == "1"
    res = None
    last_err = None
    for _attempt in range(3):
        try:
            res = run_bass_kernel_spmd(nc, in_maps, list(range(NCORES)),
                                       trace=trace)
            break
        except Exception as e:  # rare transient device errors; retry
            last_err = e
    if res is None:
        raise last_err
    global last_results
    last_results = res
    parts = []
    for i in range(NCORES):
        lo = i * per
        n = min(lo + per, T) - lo
        o = res.results[i]["out"][:n]
        parts.append(np.asarray(o, dtype=np.float32))
    return np.concatenate(parts, axis=0)


# revision 46
# speedup vs baseline: 1.0145x; 1.0145x over previous
"""ANI angular symmetry function on 8 TRN2 NeuronCores (Bass/Tile).

out[t, r*4+s] = exp(-ETA*(m-shift_r)^2) * 2*((1+cos(theta-sigma_s))/2)**ZETA
                * fcut(d0)*fcut(d1),  m=(d0+d1)/2, theta=acos(0.95*cos_angle)

Data-parallel over triples T: each core gets T/8 rows (padded).

v3 design (log-space factorization + bf16 2x outer + software pipelining):
  dot   = sum(v0*v1)                               DVE mult + reduce
  t     = 0.95*dot/sqrt((d0*d1)^2-(0.95*dot)^2)    Square/Ln/Exp
  phi   = atan(t);  cos(theta-sigma_s) = sin(phi+sigma_s)
  radial split around center c: exp(-ETA(m-s_r)^2) = E0 * exp(a_r*msum+b_r)
    E0 = exp(-ETA*m'^2) folded (log-space) into the angular factor:
    P4_s = exp(ZETA*ln(0.5c_s+0.5) + lnfcq - ETA*m'^2)        [bf16]
    R2_r = exp(a_r*msum + b_r) written as DUPLICATED PAIRS     [bf16]
  outer: out[f,r,s] = R2[f,r]*P4[f,s]; all-bf16 step-1 innermost APs
    (pairs) so TENSOR_TENSOR runs in 2x_1p mode; 2 TTs per chunk.
Emission is software-pipelined: p1(i+1) is emitted BEFORE outer(i) so the
in-order DVE queue overlaps the next super's dot-products with the current
super's ACT-heavy radial/LHP block, and the outer product lands right when
the P4 exps complete. Only two ACT table sets (natural_log_exp +
trig_and_small); set-bound ACT ops chained in program order.
"""
import math
import os
import numpy as np

ETA = 12.5
ZETA = 14.1
CUTOFF = 3.5
NCORES = 8
P = 128
CENTER = 2.1

NO_CHAIN = os.environ.get("ANI_NO_CHAIN", "0") == "1"
GPS_ARG = os.environ.get("ANI_GPS_ARG", "0") == "1"  # arg-STT on gpsimd
SQDVE = os.environ.get("ANI_SQDVE", "0") == "1"   # u^2/dot^2 on DVE
W13DVE = os.environ.get("ANI_W13DVE", "1") == "1"  # odd-section sums on DVE

_cache = {}
last_results = None  # BassKernelResults from the most recent run (for test.py)


def _geometry(T):
    per = (T + NCORES - 1) // NCORES
    # f_tot multiple of 120: supers with F_S % 40 == 0 (outer chunks of
    # F_S/10 and v-chunks of F_S/8 both integral)
    f_tot = ((per + P * 120 - 1) // (P * 120)) * 120
    return per, f_tot


def _split_supers(f_tot):
    # 4 supers, all % 40 == 0; small first (shorter DMA ramp) and smallest
    # last (shorter pipeline tail), middle two take the remainder
    q = f_tot // 40
    first = q // 4 * 40
    last = max(40, (q * 93 // 400) * 40)
    mid = f_tot - first - last
    m0 = mid // 2 // 40 * 40
    sizes = [first, m0, mid - m0, last]
    assert sum(sizes) == f_tot and all(x > 0 and x % 40 == 0 for x in sizes)
    return sizes


def _build(shifts, sections, f_tot):
    import concourse.bass as bass
    import concourse.bacc as bacc
    import concourse.tile as tile
    from concourse import mybir
    from concourse.tile_rust import add_dep_helper

    AF = mybir.ActivationFunctionType
    ALU = mybir.AluOpType
    F32 = mybir.dt.float32
    BF16 = mybir.dt.bfloat16

    sup_sizes = _split_supers(f_tot)
    NSUP = len(sup_sizes)
    assert sum(sup_sizes) == f_tot and all(x % 40 == 0 for x in sup_sizes)
    npad = P * f_tot

    SQS = math.sqrt(ETA) / 2.0
    MQ_BIAS = -math.sqrt(ETA) * CENTER
    sig = [float(x) for x in sections]
    # radial: exp(-ETA(m-s_r)^2) = exp(-ETA m'^2) * exp(a_r*msum + b_r)
    # with m = msum/2, m' = m - CENTER, delta_r = s_r - CENTER;
    # ln(0.5) folds the overall normalization into b_r.
    deltas = [float(s) - CENTER for s in shifts]
    a_r = [ETA * d for d in deltas]
    b_r = [-2.0 * ETA * d * CENTER - ETA * d * d + math.log(0.5)
           for d in deltas]

    # Force ACT table-set selection to the two sets that cover all our
    # functions; the rust pass greedily picks the first set containing a
    # func (ln->natural_log, exp->exp_and_others) which doubles the loads.
    import concourse.bacc as _bacc_mod
    from concourse.hw_specs import get_activation_tables as _real_tabs
    _KEEP = {"natural_log_exp_and_others", "trig_and_small"}

    def _filtered_tabs(arch):
        real = _real_tabs(arch)
        if not _KEEP.issubset(real.keys()):
            return real  # unexpected act_info layout: don't filter
        return {name: (fns if name in _KEEP else set())
                for name, fns in real.items()}

    _bacc_mod.get_activation_tables = _filtered_tabs

    nc = bacc.Bacc("TRN2", target_bir_lowering=False, debug=False,
                   num_devices=NCORES)

    d_dram = nc.dram_tensor("d", [2, npad], F32, kind="ExternalInput")
    v_dram = nc.dram_tensor("v", [2, npad, 3], F32, kind="ExternalInput")
    o_dram = nc.dram_tensor("out", [npad, 32], BF16, kind="ExternalOutput")

    d_v = [d_dram[i].rearrange("(p f) -> p f", p=P) for i in range(2)]
    v_v = [v_dram[i].rearrange("(p f) c -> p (f c)", p=P) for i in range(2)]
    o_v = o_dram.rearrange("(p f) k -> p (f k)", p=P)

    # chain all table-set-bound ACT ops in program order
    prev_act = [None]
    SET_BOUND = {AF.Ln, AF.Exp, AF.Sin, AF.Arctan}

    def act(out, in_, func, **kw):
        ins = nc.scalar.activation(out, in_, func, **kw)
        if func in SET_BOUND and not NO_CHAIN:
            if prev_act[0] is not None:
                add_dep_helper(ins.ins, prev_act[0].ins, sync=False,
                               reason="act-set-order")
            prev_act[0] = ins
        return ins

    with tile.TileContext(nc) as tc:
        import contextlib
        ctx = contextlib.ExitStack()
        with ctx:
            consts = ctx.enter_context(tc.tile_pool(name="consts", bufs=1))
            pers2 = ctx.enter_context(tc.tile_pool(name="pers2", bufs=2))
            po1 = ctx.enter_context(tc.tile_pool(name="po1", bufs=1))
            rp2 = ctx.enter_context(tc.tile_pool(name="rp2", bufs=2))
            pv = ctx.enter_context(tc.tile_pool(name="pv", bufs=2))
            op = ctx.enter_context(tc.tile_pool(name="op", bufs=3))

            cvals = [math.log(0.95), 0.5, math.pi / 2, MQ_BIAS, 0.0,
                     sig[0], math.pi / 2 - sig[0]]
            cvals += b_r
            cb = consts.tile([P, len(cvals)], F32, name="cb")
            for i, v in enumerate(cvals):
                nc.gpsimd.memset(cb[:, i:i + 1], v)
            (B_LN095, B_HALF, B_PI2, B_MQ, B_ZERO) = (
                cb[:, i:i + 1] for i in range(5))
            B_SIGA = cb[:, 5:6]   # sigma_0
            B_SIGB = cb[:, 6:7]   # pi/2 - sigma_0 (scale=-1 identity)
            B_RAD = [cb[:, 7 + i:8 + i] for i in range(8)]

            def emit_p1(sc, j_sup, F_S, tiles):
                """dot, t = cot(theta), msum for super sc."""
                F1 = F_S // 2
                d_sb = pers2.tile([P, 2 * F_S], F32, name="d_sb", tag="d_sb")
                nc.sync.dma_start(d_sb[:, :F_S], d_v[0][:, j_sup:j_sup + F_S])
                nc.sync.dma_start(d_sb[:, F_S:], d_v[1][:, j_sup:j_sup + F_S])
                t_sb = pers2.tile([P, F_S], F32, name="t_sb", tag="t_sb")
                msum = pers2.tile([P, F_S], F32, name="msum", tag="msum")
                dot = po1.tile([P, F_S], F32, name="dot", tag="dot")
                q2 = po1.tile([P, F_S], F32, name="q2", tag="q2")
                for i1 in range(2):
                    a, b = i1 * F1, (i1 + 1) * F1
                    v0t = pv.tile([P, 3 * F1], F32, name="v0t", tag="v0t")
                    v1t = pv.tile([P, 3 * F1], F32, name="v1t", tag="v1t")
                    nc.sync.dma_start(
                        v0t[:], v_v[0][:, 3 * (j_sup + a):3 * (j_sup + b)])
                    nc.sync.dma_start(
                        v1t[:], v_v[1][:, 3 * (j_sup + a):3 * (j_sup + b)])
                    nc.vector.tensor_tensor(v0t[:], v0t[:], v1t[:], ALU.mult)
                    nc.vector.tensor_reduce(
                        dot[:, a:b], v0t[:].rearrange("p (f c) -> p f c", c=3),
                        axis=mybir.AxisListType.X, op=ALU.add)
                d0s, d1s = d_sb[:, :F_S], d_sb[:, F_S:]
                # first super: half-grained so ACT engages before the full
                # v-DMA lands (shorter pipeline ramp)
                nh = 2 if sc == 0 else 1
                FN = F_S // nh
                for h in range(nh):
                    sl = slice(h * FN, (h + 1) * FN)
                    nc.vector.tensor_tensor(t_sb[:, sl], d0s[:, sl],
                                            d1s[:, sl], ALU.mult)  # u
                    nc.vector.tensor_tensor(msum[:, sl], d0s[:, sl],
                                            d1s[:, sl], ALU.add)
                    if SQDVE:
                        nc.vector.tensor_tensor(t_sb[:, sl], t_sb[:, sl],
                                                t_sb[:, sl], ALU.mult)
                        nc.vector.scalar_tensor_tensor(
                            q2[:, sl], dot[:, sl], 0.9025, dot[:, sl],
                            ALU.mult, ALU.mult)
                    else:
                        nc.scalar.activation(t_sb[:, sl], t_sb[:, sl],
                                             AF.Square, bias=B_ZERO)
                        nc.scalar.activation(q2[:, sl], dot[:, sl],
                                             AF.Square, bias=B_ZERO,
                                             scale=0.95)
                    # S2 = u^2 - (0.95 dot)^2  (in t_sb)
                    nc.vector.tensor_tensor(t_sb[:, sl], t_sb[:, sl],
                                            q2[:, sl], ALU.subtract)
                    act(t_sb[:, sl], t_sb[:, sl], AF.Ln, bias=B_ZERO)
                    act(t_sb[:, sl], t_sb[:, sl], AF.Exp, bias=B_LN095,
                        scale=-0.5)
                    # t = cot(theta) = 0.95*dot/sqrt(S2)
                    nc.vector.tensor_tensor(t_sb[:, sl], dot[:, sl],
                                            t_sb[:, sl], ALU.mult)
                tiles[sc] = (d_sb, t_sb, msum)

            def emit_mid_head(sc, F_S, tiles, mids):
                """trig ladder + fcq chain (independent of the deferred
                outer product, so it precedes it in every queue)."""
                d_sb, t_sb, msum = tiles[sc]
                sins = po1.tile([P, F_S, 4], F32, name="sins", tag="sins")
                P4 = rp2.tile([P, F_S, 4], BF16, name="P4", tag="P4")
                R2 = rp2.tile([P, F_S, 16], BF16, name="R2", tag="R2")
                mids[sc] = (P4, R2)
                # ---- trig set ----
                # fcut cosines first (only needs d_sb) so the fcq ops can
                # start while the Arctan/Sin ladder runs; fcq -> d_sb[:, :F]
                act(d_sb[:], d_sb[:], AF.Sin, bias=B_PI2,
                    scale=-math.pi / CUTOFF)
                act(t_sb[:], t_sb[:], AF.Arctan, bias=B_ZERO)
                # A = sin(phi+sig0) -> slot 0, B = cos(phi+sig0) -> slot 2;
                # slots 1/3 via angle addition:
                #   c1 = (A+B)*sqrt(2)/2, c3 = (B-A)*sqrt(2)/2
                # (the sqrt(2)/2 factor folds into the odd-slot Ln scale)
                act(sins[:, :, 0], t_sb[:], AF.Sin, bias=B_SIGA, scale=1.0)
                act(sins[:, :, 2], t_sb[:], AF.Sin, bias=B_SIGB, scale=-1.0)
                # mq = ETA*m'^2 -> t_sb (square is in every table set);
                # emitted after the Sins (last readers of t_sb).
                nc.scalar.activation(t_sb[:], msum[:], AF.Square, bias=B_MQ,
                                     scale=SQS)
                tiles[sc] = (d_sb, t_sb, msum, sins)

            def emit_mid_head_dve(sc, F_S, tiles):
                """fcq chain + odd-section sums (emitted between outer
                chunks of the previous super so the in-order DVE queue
                matches ready-time order)."""
                d_sb, t_sb, msum, sins = tiles[sc]
                # fcq = (1+c0)*(1+c1)  [= 4*fcut0*fcut1] -> d_sb[:, :F]
                fcq = d_sb[:, :F_S]
                nc.vector.tensor_scalar_add(d_sb[:, F_S:], d_sb[:, F_S:], 1.0)
                nc.vector.scalar_tensor_tensor(
                    fcq, fcq, 1.0, d_sb[:, F_S:], ALU.add, ALU.mult)
                w_eng = nc.gpsimd if not W13DVE else nc.vector
                w_eng.tensor_tensor(sins[:, :, 1], sins[:, :, 0],
                                    sins[:, :, 2], ALU.add)
                w_eng.tensor_tensor(sins[:, :, 3], sins[:, :, 2],
                                    sins[:, :, 0], ALU.subtract)

            def emit_mid_tail(sc, F_S, tiles, mids, split_rad=False):
                """ln/exp block: lnfcq, LHP lns, radials, args, P4 exps."""
                d_sb, t_sb, msum, sins = tiles[sc]
                P4, R2 = mids[sc]
                fcq = d_sb[:, :F_S]
                # Order: lnfcq, all Lns, then radials (no DVE deps) so ACT
                # stays busy while DVE finishes the outer and computes args.
                act(fcq, fcq, AF.Ln, bias=B_ZERO)
                FHW = F_S // 2
                SC_ODD = 0.5 * math.sqrt(0.5)
                sfl = sins[:].rearrange("p f s -> p (f s)")
                pfl = P4[:].rearrange("p f s -> p (f s)")
                for hh in range(2):
                    ha = hh * FHW
                    for par, sc_ in ((0, 0.5), (1, SC_ODD)):
                        lap = bass.AP(tensor=sfl.tensor,
                                      offset=sfl.offset + 4 * ha + par,
                                      ap=[sfl.ap[0], [4, FHW], [2, 2]])
                        act(lap, lap, AF.Ln, bias=B_HALF, scale=sc_)
                # G = ln(fcq) - ETA*m'^2
                nc.vector.scalar_tensor_tensor(
                    fcq, t_sb[:], -1.0, fcq, ALU.mult, ALU.add)

                def emit_rad(ha, n):
                    # radial pairs: R2[f, 2r:2r+2] = exp(a_r*msum + b_r)
                    rfl = R2[:].rearrange("p f k -> p (f k)")
                    for r in range(8):
                        dst = bass.AP(tensor=rfl.tensor,
                                      offset=rfl.offset + 16 * ha + 2 * r,
                                      ap=[rfl.ap[0], [16, n], [1, 2]])
                        src = bass.AP(tensor=msum.tensor,
                                      offset=msum[:].offset + ha,
                                      ap=[msum[:].ap[0], [1, n], [0, 2]])
                        act(dst, src, AF.Exp, bias=B_RAD[r], scale=a_r[r])

                def emit_arg(hh):
                    ha = hh * FHW
                    sq3 = sins[:, ha:ha + FHW, :]
                    gb = d_sb[:, ha:ha + 1]
                    g_ap = bass.AP(tensor=gb.tensor, offset=gb.offset,
                                   ap=[gb.ap[0], [1, FHW], [0, 4]])
                    arg_eng = nc.gpsimd if GPS_ARG else nc.vector
                    arg_eng.scalar_tensor_tensor(
                        sq3, sq3, ZETA, g_ap, ALU.mult, ALU.add)

                def emit_exp(hh, n):
                    ha = hh * n
                    act(pfl[:, 4 * ha:4 * (ha + n)],
                        sfl[:, 4 * ha:4 * (ha + n)], AF.Exp, bias=B_ZERO)

                emit_arg(0)
                emit_arg(1)
                if split_rad:
                    # last super: half-grained radials + quarter-grained P4
                    # exps so the outer product (and its DMA) starts
                    # mid-block, shrinking the pipeline tail
                    FQ = F_S // 4
                    emit_rad(0, FHW)
                    emit_exp(0, FQ)
                    emit_exp(1, FQ)
                    emit_rad(FHW, FHW)
                    emit_exp(2, FQ)
                    emit_exp(3, FQ)
                else:
                    emit_rad(0, F_S)
                    emit_exp(0, FHW)
                    emit_exp(1, FHW)

            def emit_outer(sc, j_sup, F_S, P4, R2, chunks=None, fine=False):
                """all-bf16 2x outer product + DMA out."""
                NCH = 10 if fine else 5
                F3 = F_S // NCH
                rfl = R2[:].rearrange("p f k -> p (f k)")
                pfl = P4[:].rearrange("p f s -> p (f s)")
                for i3 in (range(NCH) if chunks is None else chunks):
                    a = i3 * F3
                    ot = op.tile([P, F3 * 32], BF16, name="ot", tag="ot")
                    ob = ot[:]
                    for sp in range(2):
                        o_ap = bass.AP(tensor=ob.tensor,
                                       offset=ob.offset + 2 * sp,
                                       ap=[ob.ap[0], [32, F3], [4, 8],
                                           [1, 2]])
                        p_ap = bass.AP(tensor=pfl.tensor,
                                       offset=pfl.offset + 4 * a + 2 * sp,
                                       ap=[pfl.ap[0], [4, F3], [0, 8],
                                           [1, 2]])
                        r_ap = bass.AP(tensor=rfl.tensor,
                                       offset=rfl.offset + 16 * a,
                                       ap=[rfl.ap[0], [16, F3], [2, 8],
                                           [1, 2]])
                        nc.vector.tensor_tensor(o_ap, r_ap, p_ap, ALU.mult)
                    g0 = 32 * (j_sup + a)
                    # issue output stores via idle GPSIMD (SWDGE) so they
                    # don't serialize behind input-load issue on the sync
                    # sequencer
                    odma = (nc.sync if os.environ.get("ANI_ODMA") == "s"
                            else nc.gpsimd)
                    odma.dma_start(o_v[:, g0:g0 + 32 * F3], ot[:])

            tiles = {}
            mids = {}
            j_sup = 0
            offs = []
            for sc, fs in enumerate(sup_sizes):
                offs.append(j_sup)
                emit_p1(sc, j_sup, fs, tiles)
                emit_mid_head(sc, fs, tiles, mids)
                if sc > 0:
                    P4p, R2p = mids[sc - 1]
                    emit_outer(sc - 1, offs[sc - 1], sup_sizes[sc - 1],
                               P4p, R2p, chunks=range(3))
                emit_mid_head_dve(sc, fs, tiles)
                if sc > 0:
                    emit_outer(sc - 1, offs[sc - 1], sup_sizes[sc - 1],
                               P4p, R2p, chunks=range(3, 5))
                emit_mid_tail(sc, fs, tiles, mids,
                              split_rad=(sc == NSUP - 1))
                j_sup += fs
            last = NSUP - 1
            emit_outer(last, offs[last], sup_sizes[last], *mids[last],
                       fine=True)

    nc.compile()
    return nc


def kernel(tri_distances, tri_vectors, shifts, sections):
    from concourse.bass_utils import run_bass_kernel_spmd

    T = tri_distances.shape[1]
    per, f_tot = _geometry(T)
    npad = P * f_tot

    key = (T, NO_CHAIN, GPS_ARG, SQDVE, W13DVE, os.environ.get('ANI_ODMA'))
    if key not in _cache:
        _cache[key] = _build(np.asarray(shifts, np.float64),
                             np.asarray(sections, np.float64), f_tot)
    nc = _cache[key]

    d_full = np.ascontiguousarray(np.asarray(tri_distances, np.float32))
    v_full = np.ascontiguousarray(np.asarray(tri_vectors, np.float32))

    in_maps = []
    for i in range(NCORES):
        lo = i * per
        hi = min(lo + per, T)
        n = hi - lo
        dpad = np.empty((2, npad), np.float32)
        dpad[:, :n] = d_full[:, lo:hi]
        dpad[:, n:] = 1.0
        vpad = np.empty((2, npad, 3), np.float32)
        vpad[:, :n] = v_full[:, lo:hi]
        vpad[:, n:, 0] = 1.0
        vpad[:, n:, 1:] = 0.0
        in_maps.append({"d": dpad, "v": vpad})

    trace = os.environ.get("ANI_TRACE", "0") == "1"
    res = None
    last_err = None
    for _attempt in range(3):
        try:
            res = run_bass_kernel_spmd(nc, in_maps, list(range(NCORES)),
                                       trace=trace)
            break
        except Exception as e:  # rare transient device errors; retry
            last_err = e
    if res is None:
        raise last_err
    global last_results
    last_results = res
    parts = []
    for i in range(NCORES):
        lo = i * per
        n = min(lo + per, T) - lo
        o = res.results[i]["out"][:n]
        parts.append(np.asarray(o, dtype=np.float32))
    return np.concatenate(parts, axis=0)


# revision 48
# speedup vs baseline: 1.0159x; 1.0014x over previous
"""ANI angular symmetry function on 8 TRN2 NeuronCores (Bass/Tile).

out[t, r*4+s] = exp(-ETA*(m-shift_r)^2) * 2*((1+cos(theta-sigma_s))/2)**ZETA
                * fcut(d0)*fcut(d1),  m=(d0+d1)/2, theta=acos(0.95*cos_angle)

Data-parallel over triples T: each core gets T/8 rows (padded).

v3 design (log-space factorization + bf16 2x outer + software pipelining):
  dot   = sum(v0*v1)                               DVE mult + reduce
  t     = 0.95*dot/sqrt((d0*d1)^2-(0.95*dot)^2)    Square/Ln/Exp
  phi   = atan(t);  cos(theta-sigma_s) = sin(phi+sigma_s)
  radial split around center c: exp(-ETA(m-s_r)^2) = E0 * exp(a_r*msum+b_r)
    E0 = exp(-ETA*m'^2) folded (log-space) into the angular factor:
    P4_s = exp(ZETA*ln(0.5c_s+0.5) + lnfcq - ETA*m'^2)        [bf16]
    R2_r = exp(a_r*msum + b_r) written as DUPLICATED PAIRS     [bf16]
  outer: out[f,r,s] = R2[f,r]*P4[f,s]; all-bf16 step-1 innermost APs
    (pairs) so TENSOR_TENSOR runs in 2x_1p mode; 2 TTs per chunk.
Emission is software-pipelined: p1(i+1) is emitted BEFORE outer(i) so the
in-order DVE queue overlaps the next super's dot-products with the current
super's ACT-heavy radial/LHP block, and the outer product lands right when
the P4 exps complete. Only two ACT table sets (natural_log_exp +
trig_and_small); set-bound ACT ops chained in program order.
"""
import math
import os
import numpy as np

ETA = 12.5
ZETA = 14.1
CUTOFF = 3.5
NCORES = 8
P = 128
CENTER = 2.1

NO_CHAIN = os.environ.get("ANI_NO_CHAIN", "0") == "1"
GPS_ARG = os.environ.get("ANI_GPS_ARG", "0") == "1"  # arg-STT on gpsimd
SQDVE = os.environ.get("ANI_SQDVE", "0") == "1"   # u^2/dot^2 on DVE
W13DVE = os.environ.get("ANI_W13DVE", "1") == "1"  # odd-section sums on DVE

_cache = {}
last_results = None  # BassKernelResults from the most recent run (for test.py)


def _geometry(T):
    per = (T + NCORES - 1) // NCORES
    # f_tot multiple of 120: supers with F_S % 40 == 0 (outer chunks of
    # F_S/10 and v-chunks of F_S/8 both integral)
    f_tot = ((per + P * 120 - 1) // (P * 120)) * 120
    return per, f_tot


def _split_supers(f_tot):
    # 4 supers, all % 40 == 0; small first (shorter DMA ramp) and smallest
    # last (shorter pipeline tail), middle two take the remainder
    q = f_tot // 40
    first = q // 4 * 40
    last = max(40, (q * 93 // 400) * 40)
    mid = f_tot - first - last
    m0 = mid // 2 // 40 * 40
    sizes = [first, m0, mid - m0, last]
    assert sum(sizes) == f_tot and all(x > 0 and x % 40 == 0 for x in sizes)
    return sizes


def _build(shifts, sections, f_tot):
    import concourse.bass as bass
    import concourse.bacc as bacc
    import concourse.tile as tile
    from concourse import mybir
    from concourse.tile_rust import add_dep_helper

    AF = mybir.ActivationFunctionType
    ALU = mybir.AluOpType
    F32 = mybir.dt.float32
    BF16 = mybir.dt.bfloat16

    sup_sizes = _split_supers(f_tot)
    NSUP = len(sup_sizes)
    assert sum(sup_sizes) == f_tot and all(x % 40 == 0 for x in sup_sizes)
    npad = P * f_tot

    SQS = math.sqrt(ETA) / 2.0
    MQ_BIAS = -math.sqrt(ETA) * CENTER
    sig = [float(x) for x in sections]
    # radial: exp(-ETA(m-s_r)^2) = exp(-ETA m'^2) * exp(a_r*msum + b_r)
    # with m = msum/2, m' = m - CENTER, delta_r = s_r - CENTER;
    # ln(0.5) folds the overall normalization into b_r.
    deltas = [float(s) - CENTER for s in shifts]
    a_r = [ETA * d for d in deltas]
    b_r = [-2.0 * ETA * d * CENTER - ETA * d * d + math.log(0.5)
           for d in deltas]

    # Force ACT table-set selection to the two sets that cover all our
    # functions; the rust pass greedily picks the first set containing a
    # func (ln->natural_log, exp->exp_and_others) which doubles the loads.
    import concourse.bacc as _bacc_mod
    from concourse.hw_specs import get_activation_tables as _real_tabs
    _KEEP = {"natural_log_exp_and_others", "trig_and_small"}

    def _filtered_tabs(arch):
        real = _real_tabs(arch)
        if not _KEEP.issubset(real.keys()):
            return real  # unexpected act_info layout: don't filter
        return {name: (fns if name in _KEEP else set())
                for name, fns in real.items()}

    _bacc_mod.get_activation_tables = _filtered_tabs

    nc = bacc.Bacc("TRN2", target_bir_lowering=False, debug=False,
                   num_devices=NCORES)

    d_dram = nc.dram_tensor("d", [2, npad], F32, kind="ExternalInput")
    v_dram = nc.dram_tensor("v", [2, npad, 3], F32, kind="ExternalInput")
    o_dram = nc.dram_tensor("out", [npad, 32], BF16, kind="ExternalOutput")

    d_v = [d_dram[i].rearrange("(p f) -> p f", p=P) for i in range(2)]
    v_v = [v_dram[i].rearrange("(p f) c -> p (f c)", p=P) for i in range(2)]
    o_v = o_dram.rearrange("(p f) k -> p (f k)", p=P)

    # chain all table-set-bound ACT ops in program order
    prev_act = [None]
    SET_BOUND = {AF.Ln, AF.Exp, AF.Sin, AF.Arctan}

    def act(out, in_, func, **kw):
        ins = nc.scalar.activation(out, in_, func, **kw)
        if func in SET_BOUND and not NO_CHAIN:
            if prev_act[0] is not None:
                add_dep_helper(ins.ins, prev_act[0].ins, sync=False,
                               reason="act-set-order")
            prev_act[0] = ins
        return ins

    with tile.TileContext(nc) as tc:
        import contextlib
        ctx = contextlib.ExitStack()
        with ctx:
            consts = ctx.enter_context(tc.tile_pool(name="consts", bufs=1))
            pers2 = ctx.enter_context(tc.tile_pool(name="pers2", bufs=2))
            po1 = ctx.enter_context(tc.tile_pool(name="po1", bufs=1))
            rp2 = ctx.enter_context(tc.tile_pool(name="rp2", bufs=2))
            pv = ctx.enter_context(tc.tile_pool(name="pv", bufs=2))
            op = ctx.enter_context(tc.tile_pool(name="op", bufs=3))

            cvals = [math.log(0.95), 0.5, math.pi / 2, MQ_BIAS, 0.0,
                     sig[0], math.pi / 2 - sig[0]]
            cvals += b_r
            cb = consts.tile([P, len(cvals)], F32, name="cb")
            for i, v in enumerate(cvals):
                nc.gpsimd.memset(cb[:, i:i + 1], v)
            (B_LN095, B_HALF, B_PI2, B_MQ, B_ZERO) = (
                cb[:, i:i + 1] for i in range(5))
            B_SIGA = cb[:, 5:6]   # sigma_0
            B_SIGB = cb[:, 6:7]   # pi/2 - sigma_0 (scale=-1 identity)
            B_RAD = [cb[:, 7 + i:8 + i] for i in range(8)]

            def emit_p1(sc, j_sup, F_S, tiles):
                """dot, t = cot(theta), msum for super sc."""
                F1 = F_S // 2
                d_sb = pers2.tile([P, 2 * F_S], F32, name="d_sb", tag="d_sb")
                nc.sync.dma_start(d_sb[:, :F_S], d_v[0][:, j_sup:j_sup + F_S])
                nc.sync.dma_start(d_sb[:, F_S:], d_v[1][:, j_sup:j_sup + F_S])
                t_sb = pers2.tile([P, F_S], F32, name="t_sb", tag="t_sb")
                msum = pers2.tile([P, F_S], F32, name="msum", tag="msum")
                dot = po1.tile([P, F_S], F32, name="dot", tag="dot")
                q2 = po1.tile([P, F_S], F32, name="q2", tag="q2")
                for i1 in range(2):
                    a, b = i1 * F1, (i1 + 1) * F1
                    v0t = pv.tile([P, 3 * F1], F32, name="v0t", tag="v0t")
                    v1t = pv.tile([P, 3 * F1], F32, name="v1t", tag="v1t")
                    nc.sync.dma_start(
                        v0t[:], v_v[0][:, 3 * (j_sup + a):3 * (j_sup + b)])
                    nc.sync.dma_start(
                        v1t[:], v_v[1][:, 3 * (j_sup + a):3 * (j_sup + b)])
                    nc.vector.tensor_tensor(v0t[:], v0t[:], v1t[:], ALU.mult)
                    nc.vector.tensor_reduce(
                        dot[:, a:b], v0t[:].rearrange("p (f c) -> p f c", c=3),
                        axis=mybir.AxisListType.X, op=ALU.add)
                d0s, d1s = d_sb[:, :F_S], d_sb[:, F_S:]
                # first super: half-grained so ACT engages before the full
                # v-DMA lands (shorter pipeline ramp)
                nh = 2 if sc == 0 else 1
                FN = F_S // nh
                for h in range(nh):
                    sl = slice(h * FN, (h + 1) * FN)
                    nc.vector.tensor_tensor(t_sb[:, sl], d0s[:, sl],
                                            d1s[:, sl], ALU.mult)  # u
                    nc.vector.tensor_tensor(msum[:, sl], d0s[:, sl],
                                            d1s[:, sl], ALU.add)
                    if SQDVE:
                        nc.vector.tensor_tensor(t_sb[:, sl], t_sb[:, sl],
                                                t_sb[:, sl], ALU.mult)
                        nc.vector.scalar_tensor_tensor(
                            q2[:, sl], dot[:, sl], 0.9025, dot[:, sl],
                            ALU.mult, ALU.mult)
                    else:
                        nc.scalar.activation(t_sb[:, sl], t_sb[:, sl],
                                             AF.Square, bias=B_ZERO)
                        nc.scalar.activation(q2[:, sl], dot[:, sl],
                                             AF.Square, bias=B_ZERO,
                                             scale=0.95)
                    # S2 = u^2 - (0.95 dot)^2  (in t_sb)
                    nc.vector.tensor_tensor(t_sb[:, sl], t_sb[:, sl],
                                            q2[:, sl], ALU.subtract)
                    act(t_sb[:, sl], t_sb[:, sl], AF.Ln, bias=B_ZERO)
                    act(t_sb[:, sl], t_sb[:, sl], AF.Exp, bias=B_LN095,
                        scale=-0.5)
                    # t = cot(theta) = 0.95*dot/sqrt(S2)
                    nc.vector.tensor_tensor(t_sb[:, sl], dot[:, sl],
                                            t_sb[:, sl], ALU.mult)
                tiles[sc] = (d_sb, t_sb, msum)

            def emit_mid_head(sc, F_S, tiles, mids):
                """trig ladder + fcq chain (independent of the deferred
                outer product, so it precedes it in every queue)."""
                d_sb, t_sb, msum = tiles[sc]
                sins = po1.tile([P, F_S, 4], F32, name="sins", tag="sins")
                P4 = rp2.tile([P, F_S, 4], BF16, name="P4", tag="P4")
                R2 = rp2.tile([P, F_S, 16], BF16, name="R2", tag="R2")
                mids[sc] = (P4, R2)
                # ---- trig set ----
                # fcut cosines first (only needs d_sb) so the fcq ops can
                # start while the Arctan/Sin ladder runs; fcq -> d_sb[:, :F]
                act(d_sb[:], d_sb[:], AF.Sin, bias=B_PI2,
                    scale=-math.pi / CUTOFF)
                act(t_sb[:], t_sb[:], AF.Arctan, bias=B_ZERO)
                # A = sin(phi+sig0) -> slot 0, B = cos(phi+sig0) -> slot 2;
                # slots 1/3 via angle addition:
                #   c1 = (A+B)*sqrt(2)/2, c3 = (B-A)*sqrt(2)/2
                # (the sqrt(2)/2 factor folds into the odd-slot Ln scale)
                act(sins[:, :, 0], t_sb[:], AF.Sin, bias=B_SIGA, scale=1.0)
                act(sins[:, :, 2], t_sb[:], AF.Sin, bias=B_SIGB, scale=-1.0)
                # mq = ETA*m'^2 -> t_sb (square is in every table set);
                # emitted after the Sins (last readers of t_sb).
                nc.scalar.activation(t_sb[:], msum[:], AF.Square, bias=B_MQ,
                                     scale=SQS)
                tiles[sc] = (d_sb, t_sb, msum, sins)

            def emit_mid_head_dve(sc, F_S, tiles):
                """fcq chain + odd-section sums (emitted between outer
                chunks of the previous super so the in-order DVE queue
                matches ready-time order)."""
                d_sb, t_sb, msum, sins = tiles[sc]
                # fcq = (1+c0)*(1+c1)  [= 4*fcut0*fcut1] -> d_sb[:, :F]
                fcq = d_sb[:, :F_S]
                nc.vector.tensor_scalar_add(d_sb[:, F_S:], d_sb[:, F_S:], 1.0)
                nc.vector.scalar_tensor_tensor(
                    fcq, fcq, 1.0, d_sb[:, F_S:], ALU.add, ALU.mult)
                w_eng = nc.gpsimd if not W13DVE else nc.vector
                w_eng.tensor_tensor(sins[:, :, 1], sins[:, :, 0],
                                    sins[:, :, 2], ALU.add)
                w_eng.tensor_tensor(sins[:, :, 3], sins[:, :, 2],
                                    sins[:, :, 0], ALU.subtract)

            def emit_mid_tail(sc, F_S, tiles, mids, split_rad=False):
                """ln/exp block: lnfcq, LHP lns, radials, args, P4 exps."""
                d_sb, t_sb, msum, sins = tiles[sc]
                P4, R2 = mids[sc]
                fcq = d_sb[:, :F_S]
                # Order: lnfcq, all Lns, then radials (no DVE deps) so ACT
                # stays busy while DVE finishes the outer and computes args.
                act(fcq, fcq, AF.Ln, bias=B_ZERO)
                FHW = F_S // 2
                SC_ODD = 0.5 * math.sqrt(0.5)
                sfl = sins[:].rearrange("p f s -> p (f s)")
                pfl = P4[:].rearrange("p f s -> p (f s)")
                for hh in range(2):
                    ha = hh * FHW
                    for par, sc_ in ((0, 0.5), (1, SC_ODD)):
                        lap = bass.AP(tensor=sfl.tensor,
                                      offset=sfl.offset + 4 * ha + par,
                                      ap=[sfl.ap[0], [4, FHW], [2, 2]])
                        act(lap, lap, AF.Ln, bias=B_HALF, scale=sc_)
                # G = ln(fcq) - ETA*m'^2
                nc.vector.scalar_tensor_tensor(
                    fcq, t_sb[:], -1.0, fcq, ALU.mult, ALU.add)

                def emit_rad(ha, n):
                    # radial pairs: R2[f, 2r:2r+2] = exp(a_r*msum + b_r)
                    rfl = R2[:].rearrange("p f k -> p (f k)")
                    for r in range(8):
                        dst = bass.AP(tensor=rfl.tensor,
                                      offset=rfl.offset + 16 * ha + 2 * r,
                                      ap=[rfl.ap[0], [16, n], [1, 2]])
                        src = bass.AP(tensor=msum.tensor,
                                      offset=msum[:].offset + ha,
                                      ap=[msum[:].ap[0], [1, n], [0, 2]])
                        act(dst, src, AF.Exp, bias=B_RAD[r], scale=a_r[r])

                def emit_arg(hh, n):
                    ha = hh * n
                    sq3 = sins[:, ha:ha + n, :]
                    gb = d_sb[:, ha:ha + 1]
                    g_ap = bass.AP(tensor=gb.tensor, offset=gb.offset,
                                   ap=[gb.ap[0], [1, n], [0, 4]])
                    arg_eng = nc.gpsimd if GPS_ARG else nc.vector
                    arg_eng.scalar_tensor_tensor(
                        sq3, sq3, ZETA, g_ap, ALU.mult, ALU.add)

                def emit_exp(hh, n):
                    ha = hh * n
                    act(pfl[:, 4 * ha:4 * (ha + n)],
                        sfl[:, 4 * ha:4 * (ha + n)], AF.Exp, bias=B_ZERO)

                if split_rad:
                    # last super: quarter-grained radials/args/P4 exps so
                    # the outer product (and its DMA) trails the ACT
                    # stream quarter-by-quarter, shrinking the tail
                    FQ = F_S // 4
                    for qq in range(4):
                        emit_arg(qq, FQ)
                        emit_rad(qq * FQ, FQ)
                        emit_exp(qq, FQ)
                else:
                    emit_arg(0, FHW)
                    emit_arg(1, FHW)
                    emit_rad(0, F_S)
                    emit_exp(0, FHW)
                    emit_exp(1, FHW)

            def emit_outer(sc, j_sup, F_S, P4, R2, chunks=None, fine=False):
                """all-bf16 2x outer product + DMA out."""
                NCH = 10 if fine else 5
                F3 = F_S // NCH
                rfl = R2[:].rearrange("p f k -> p (f k)")
                pfl = P4[:].rearrange("p f s -> p (f s)")
                for i3 in (range(NCH) if chunks is None else chunks):
                    a = i3 * F3
                    ot = op.tile([P, F3 * 32], BF16, name="ot", tag="ot")
                    ob = ot[:]
                    for sp in range(2):
                        o_ap = bass.AP(tensor=ob.tensor,
                                       offset=ob.offset + 2 * sp,
                                       ap=[ob.ap[0], [32, F3], [4, 8],
                                           [1, 2]])
                        p_ap = bass.AP(tensor=pfl.tensor,
                                       offset=pfl.offset + 4 * a + 2 * sp,
                                       ap=[pfl.ap[0], [4, F3], [0, 8],
                                           [1, 2]])
                        r_ap = bass.AP(tensor=rfl.tensor,
                                       offset=rfl.offset + 16 * a,
                                       ap=[rfl.ap[0], [16, F3], [2, 8],
                                           [1, 2]])
                        nc.vector.tensor_tensor(o_ap, r_ap, p_ap, ALU.mult)
                    g0 = 32 * (j_sup + a)
                    # issue output stores via idle GPSIMD (SWDGE) so they
                    # don't serialize behind input-load issue on the sync
                    # sequencer
                    odma = (nc.sync if os.environ.get("ANI_ODMA") == "s"
                            else nc.gpsimd)
                    odma.dma_start(o_v[:, g0:g0 + 32 * F3], ot[:])

            tiles = {}
            mids = {}
            j_sup = 0
            offs = []
            for sc, fs in enumerate(sup_sizes):
                offs.append(j_sup)
                emit_p1(sc, j_sup, fs, tiles)
                emit_mid_head(sc, fs, tiles, mids)
                if sc > 0:
                    P4p, R2p = mids[sc - 1]
                    emit_outer(sc - 1, offs[sc - 1], sup_sizes[sc - 1],
                               P4p, R2p, chunks=range(3))
                emit_mid_head_dve(sc, fs, tiles)
                if sc > 0:
                    emit_outer(sc - 1, offs[sc - 1], sup_sizes[sc - 1],
                               P4p, R2p, chunks=range(3, 5))
                emit_mid_tail(sc, fs, tiles, mids,
                              split_rad=(sc == NSUP - 1))
                j_sup += fs
            last = NSUP - 1
            emit_outer(last, offs[last], sup_sizes[last], *mids[last],
                       fine=True)

    nc.compile()
    return nc


def kernel(tri_distances, tri_vectors, shifts, sections):
    from concourse.bass_utils import run_bass_kernel_spmd

    T = tri_distances.shape[1]
    per, f_tot = _geometry(T)
    npad = P * f_tot

    key = (T, NO_CHAIN, GPS_ARG, SQDVE, W13DVE, os.environ.get('ANI_ODMA'))
    if key not in _cache:
        _cache[key] = _build(np.asarray(shifts, np.float64),
                             np.asarray(sections, np.float64), f_tot)
    nc = _cache[key]

    d_full = np.ascontiguousarray(np.asarray(tri_distances, np.float32))
    v_full = np.ascontiguousarray(np.asarray(tri_vectors, np.float32))

    in_maps = []
    for i in range(NCORES):
        lo = i * per
        hi = min(lo + per, T)
        n = hi - lo
        dpad = np.empty((2, npad), np.float32)
        dpad[:, :n] = d_full[:, lo:hi]
        dpad[:, n:] = 1.0
        vpad = np.empty((2, npad, 3), np.float32)
        vpad[:, :n] = v_full[:, lo:hi]
        vpad[:, n:, 0] = 1.0
        vpad[:, n:, 1:] = 0.0
        in_maps.append({"d": dpad, "v": vpad})

    trace = os.environ.get("ANI_TRACE", "0") == "1"
    res = None
    last_err = None
    for _attempt in range(3):
        try:
            res = run_bass_kernel_spmd(nc, in_maps, list(range(NCORES)),
                                       trace=trace)
            break
        except Exception as e:  # rare transient device errors; retry
            last_err = e
    if res is None:
        raise last_err
    global last_results
    last_results = res
    parts = []
    for i in range(NCORES):
        lo = i * per
        n = min(lo + per, T) - lo
        o = res.results[i]["out"][:n]
        parts.append(np.asarray(o, dtype=np.float32))
    return np.concatenate(parts, axis=0)
